# revision 52
# baseline (speedup 1.0000x reference)
"""BroadcastAttention Trainium2 kernel (8 NeuronCores, data-parallel over batch).

Math per sample (C=512, N=4096, H=8 heads, HD=64):
    qkv = Wqkv @ x            # [H*(1+2HD), N]
    q[h,n], k[h,d,n], v[h,d,n] split per head
    s = softmax(q over n)     # [H, N]
    ctx[h,d] = sum_n k[h,d,n]*s[h,n]
    out = Wp @ (relu(v)*ctx) + bp

Device formulation (per core: 2 samples):
    - kT[n, kch] and expT[n, h] = exp(qT[n, h]) computed directly in
      [n-partition] orientation (softmax max-shift is unnecessary: |q|<~3,
      exp is safely in fp32 range; exp(q)/sum(exp q) == softmax exactly).
    - ctx2[h, kch] = sum_n expT[n,h] * kT[n,kch] via PE matmul accumulation;
      Z[h] = sum_n expT[n,h] via matmul with ones; ctx = ctx2 * (1/Z).
    - ctx transposed to [kch-partition] via PE transpose, head-selected with a
      0/1 mask, then fused into the PSUM->SBUF eviction of v as
      a = max(v,0)*ctx (DVE tensor_scalar).
    - proj matmul over a, bias-add on ScalarE during PSUM->SBUF eviction.

Matmul operands are bf16 (fp32 streams through the PE at half rate; measured
550ns vs ~220ns per 512-col matmul), accumulation in fp32 PSUM.
"""

import sys

for _p in ("/opt/trn_rl_repo",):
    if _p not in sys.path:
        sys.path.insert(0, _p)

from contextlib import ExitStack

import ml_dtypes
import numpy as np

import concourse.bass as bass
import concourse.mybir as mybir
import concourse.tile as tile
from concourse import bacc
from concourse.bass_utils import run_bass_kernel_spmd
from concourse.masks import make_identity

# Problem constants (hardcoded per contract; kernel.py must be self-contained).
B, C, N = 16, 512, 4096
H, HD = 8, 64
NCORES = 8
BPC = B // NCORES  # samples per core
CT = C // 128      # 4 contraction/partition tiles of 128
NT = N // 128      # 32 n-tiles (kT orientation)
FREE = 512         # matmul moving free-dim chunk
NCH = N // FREE    # 8 chunks
FP = mybir.dt.float32
BF = mybir.dt.bfloat16  # matmul operand dtype (fp32 PSUM accumulation)

# Results of the last run (for test harness introspection).
LAST_RESULTS = None


def _build(has_qkv_bias: bool, has_p_bias: bool) -> bass.Bass:
    nc = bacc.Bacc("TRN2", target_bir_lowering=False, debug=False)

    x_d = nc.declare_dram_parameter("x", [BPC, C, N], FP, isOutput=False)
    wk_d = nc.declare_dram_parameter("wkT", [C, C], BF, isOutput=False)
    wv_d = nc.declare_dram_parameter("wvT", [C, C], BF, isOutput=False)
    wp_d = nc.declare_dram_parameter("wpT", [C, C], BF, isOutput=False)
    wq_d = nc.declare_dram_parameter("wqT", [C, H], BF, isOutput=False)
    mask_d = nc.declare_dram_parameter("maskhd", [C, H], FP, isOutput=False)
    esel_d = nc.declare_dram_parameter("esel", [128, H], FP, isOutput=False)
    bq_d = nc.declare_dram_parameter("bq", [1, H], BF, isOutput=False)
    bk_d = nc.declare_dram_parameter("bk", [1, C], BF, isOutput=False)
    bv_d = nc.declare_dram_parameter("bv", [1, C], BF, isOutput=False)
    bp_d = nc.declare_dram_parameter("bp", [C], FP, isOutput=False)
    y_d = nc.declare_dram_parameter("y", [BPC, C, N], FP, isOutput=True)

    AF = mybir.ActivationFunctionType
    OP = mybir.AluOpType

    with tile.TileContext(nc) as tc, ExitStack() as ctx:
        consts = ctx.enter_context(tc.tile_pool(name="consts", bufs=1))
        xstage = ctx.enter_context(tc.tile_pool(name="xstage", bufs=4))
        xpool = ctx.enter_context(tc.tile_pool(name="xpool", bufs=2))
        apool = ctx.enter_context(tc.tile_pool(name="apool", bufs=1))
        spool = ctx.enter_context(tc.tile_pool(name="spool", bufs=2))
        ktpool = ctx.enter_context(tc.tile_pool(name="ktpool", bufs=10))
        opool = ctx.enter_context(tc.tile_pool(name="opool", bufs=3))
        small = ctx.enter_context(tc.tile_pool(name="small", bufs=2))
        ps_mm = ctx.enter_context(tc.tile_pool(name="ps_mm", bufs=4, space="PSUM"))
        ps_q = ctx.enter_context(tc.tile_pool(name="ps_q", bufs=2, space="PSUM"))
        ps_ctx = ctx.enter_context(tc.tile_pool(name="ps_ctx", bufs=1, space="PSUM"))
        ps_z = ctx.enter_context(tc.tile_pool(name="ps_z", bufs=1, space="PSUM"))

        # ---- constants / weights into SBUF ----
        wk_sb = consts.tile([128, CT, C], BF)
        wv_sb = consts.tile([128, CT, C], BF)
        wp_sb = consts.tile([128, CT, C], BF)
        wq_sb = consts.tile([128, CT, H], BF)
        mask_sb = consts.tile([128, CT, H], FP)
        bp_sb = consts.tile([128, CT], FP)
        # Only the QK-phase weights load up front (GpSimd queue, away from x
        # on Sync). wv/wp/mask/bp are not needed until the V/P phases —
        # issuing them on Sync *after* batch 0's x chunks keeps the first
        # ~25us of HBM bandwidth for x (the startup critical path).
        for ct in range(CT):
            sl = slice(ct * 128, (ct + 1) * 128)
            nc.gpsimd.dma_start(out=wk_sb[:, ct, :], in_=wk_d[sl, :])
            nc.gpsimd.dma_start(out=wq_sb[:, ct, :], in_=wq_d[sl, :])

        identity = consts.tile([128, 128], FP)
        make_identity(nc, identity[:])
        ones_col = consts.tile([128, 1], FP)
        nc.vector.memset(ones_col[:], 1.0)
        esel_sb = consts.tile([128, H], FP)
        nc.gpsimd.dma_start(out=esel_sb[:], in_=esel_d[:, :])

        if has_qkv_bias:
            bq_sb = consts.tile([1, H], BF)
            bk_sb = consts.tile([1, C], BF)
            bv_sb = consts.tile([1, C], BF)
            ones_row = consts.tile([1, FREE], BF)
            nc.sync.dma_start(out=bq_sb[:], in_=bq_d[:, :])
            nc.sync.dma_start(out=bk_sb[:], in_=bk_d[:, :])
            nc.sync.dma_start(out=bv_sb[:], in_=bv_d[:, :])
            nc.vector.memset(ones_row[:], 1.0)

        for b in range(BPC):
            # ---- load x[b] (fp32) and convert to bf16 ----
            # Load x in n-quarters (quarter-major) so the QK loop can start
            # after ~1/4 of the 8.4MB load instead of all of it; conversions
            # alternate ScalarE/VectorE so neither FIFO stalls the QK path.
            # All conversions live on ScalarE: DVE is the busier engine
            # (PSUM evictions), and ScalarE has slack.
            x_sb = xpool.tile([128, CT, N], BF, tag="x_sb", name="x_sb")
            NQ = 8 if b == 0 else 4  # fine chunks only where start latency matters
            QN = N // NQ
            for qd in range(NQ):
                nsl = slice(qd * QN, (qd + 1) * QN)
                for ct in range(CT):
                    xst = xstage.tile([128, QN], FP, tag="xst", name="xst")
                    nc.sync.dma_start(
                        out=xst[:], in_=x_d[b, ct * 128:(ct + 1) * 128, nsl]
                    )
                    if (qd * CT + ct) % 2 == 0:
                        nc.scalar.copy(x_sb[:, ct, nsl], xst[:])
                    else:
                        nc.vector.tensor_copy(x_sb[:, ct, nsl], xst[:])
            if b == 0:
                for ct in range(CT):
                    sl = slice(ct * 128, (ct + 1) * 128)
                    nc.sync.dma_start(out=wv_sb[:, ct, :], in_=wv_d[sl, :])
                    nc.sync.dma_start(out=wp_sb[:, ct, :], in_=wp_d[sl, :])
                    nc.sync.dma_start(out=mask_sb[:, ct, :], in_=mask_d[sl, :])
                nc.sync.dma_start(
                    out=bp_sb[:], in_=bp_d.rearrange("(o p) -> p o", p=128)
                )

            # ---- QK phase: kT / expT per n-tile, ctx & Z accumulation ----
            # ctx matmuls have M=8 (15/16 of the PE array idle), so they are
            # issued back-to-back in groups of 4 at distinct 32-column groups
            # (tile_position) to run concurrently; ctx_big[32j+h] holds the
            # partial sum for sub-stream j, combined afterwards with one
            # matmul against the constant selection matrix esel.
            scoresT = spool.tile([128, NT, H], BF, tag="scoresT", name="scoresT")
            ctx_big = ps_ctx.tile([128, C], FP, tag="ctx", name="ctx_big")
            # Zero the whole bank: the M=8 col-tiled matmuls only write rows
            # 32j..32j+7, but the esel combine matmul reads all 128 rows.
            nc.vector.memset(ctx_big[:], 0.0)
            def emit_ctx_group(ntg, kts):
                for j in range(4):
                    nt = ntg * 4 + j
                    nc.tensor.matmul(
                        ctx_big[32 * j:32 * j + H, :],
                        scoresT[:, nt, :], kts[j][:],
                        start=False, stop=(ntg == NT // 4 - 1),
                        skip_group_check=True,
                        tile_position=(0, 32 * j),
                    )

            # ctx matmul groups are issued one group late (software pipeline)
            # so their kt/scoresT inputs are already evicted from PSUM by the
            # time PE reaches them — no PE stall on the DVE copies.
            prev = None
            for ntg in range(NT // 4):
                kts = []
                for j in range(4):
                    nt = ntg * 4 + j
                    k_ps = ps_mm.tile([128, C], FP, tag="mm512", name="k_ps")
                    q_ps = ps_q.tile([128, H], FP, tag="q8", name="q_ps")
                    nsl = slice(nt * 128, (nt + 1) * 128)
                    for ct in range(CT):
                        xsl = x_sb[:, ct, nsl]
                        last = (ct == CT - 1) and not has_qkv_bias
                        nc.tensor.matmul(
                            k_ps[:], xsl, wk_sb[:, ct, :],
                            start=(ct == 0), stop=last,
                        )
                        nc.tensor.matmul(
                            q_ps[:], xsl, wq_sb[:, ct, :],
                            start=(ct == 0), stop=last,
                        )
                    if has_qkv_bias:
                        onesl = ones_row[:, 0:128]
                        nc.tensor.matmul(k_ps[:], onesl, bk_sb[:, :], start=False, stop=True)
                        nc.tensor.matmul(q_ps[:], onesl, bq_sb[:, :], start=False, stop=True)
                    st = scoresT[:, nt, :]
                    nc.scalar.activation(out=st, in_=q_ps[:], func=AF.Exp)
                    kt_sb = ktpool.tile([128, C], BF, tag="kt", name="kt_sb")
                    nc.vector.tensor_copy(out=kt_sb[:, :C // 2], in_=k_ps[:, :C // 2])
                    nc.scalar.copy(kt_sb[:, C // 2:], k_ps[:, C // 2:])
                    kts.append(kt_sb)
                if prev is not None:
                    emit_ctx_group(ntg - 1, prev)
                prev = kts
            emit_ctx_group(NT // 4 - 1, prev)

            # ---- ctx finalize: combine sub-streams, normalize, transpose,
            # head-select ----
            ctxcopy = small.tile([128, C], FP, tag="ctxcopy", name="ctxcopy")
            nc.vector.tensor_copy(out=ctxcopy[:], in_=ctx_big[:])
            ctx2_ps = ps_q.tile([H, C], FP, tag="q8", name="ctx2_ps")
            nc.tensor.matmul(ctx2_ps[:], esel_sb[:], ctxcopy[:], start=True, stop=True)
            # Z[h] = sum_n exp(q): per-partition partial on DVE, then one tiny
            # cross-partition matmul (cheaper than 64 PE matmuls).
            zpart = small.tile([128, H], FP, tag="zpart", name="zpart")
            nc.vector.reduce_sum(
                out=zpart[:],
                in_=scoresT[:].rearrange("p nt h -> p h nt"),
                axis=mybir.AxisListType.X,
            )
            z_ps = ps_z.tile([H, 1], FP, tag="z", name="z_ps")
            nc.tensor.matmul(z_ps[:], zpart[:], ones_col[:], start=True, stop=True)
            invz = small.tile([H, 1], FP, tag="invz", name="invz")
            nc.vector.reciprocal(out=invz[:], in_=z_ps[:])
            ctxn = small.tile([H, C], FP, tag="ctxn", name="ctxn")
            nc.vector.tensor_scalar_mul(out=ctxn[:], in0=ctx2_ps[:], scalar1=invz[:])
            ctxv = small.tile([128, CT], FP, tag="ctxv", name="ctxv")
            for i in range(CT):
                ctxT_ps = ps_q.tile([128, H], FP, tag="q8", name="ctxT_ps")
                nc.tensor.transpose(
                    ctxT_ps[:], ctxn[:, i * 128:(i + 1) * 128], identity[:H, :H]
                )
                junk = small.tile([128, H], FP, tag="junk", name="junk")
                nc.vector.tensor_tensor(
                    out=junk[:], in0=ctxT_ps[:], in1=mask_sb[:, i, :], op=OP.mult
                )
                nc.vector.reduce_sum(
                    out=ctxv[:, i:i + 1], in_=junk[:], axis=mybir.AxisListType.X
                )

            # ---- V phase: v matmul + fused relu*ctx eviction ----
            a_sb = apool.tile([128, CT, N], BF, tag="a_sb", name="a_sb")
            for i in range(CT):
                for chk in range(NCH):
                    v_ps = ps_mm.tile([128, FREE], FP, tag="mm512", name="v_ps")
                    csl = slice(chk * FREE, (chk + 1) * FREE)
                    for ct in range(CT):
                        last = (ct == CT - 1) and not has_qkv_bias
                        nc.tensor.matmul(
                            v_ps[:],
                            wv_sb[:, ct, i * 128:(i + 1) * 128],
                            x_sb[:, ct, csl],
                            start=(ct == 0), stop=last,
                        )
                    if has_qkv_bias:
                        nc.tensor.matmul(
                            v_ps[:], bv_sb[:, i * 128:(i + 1) * 128], ones_row[:],
                            start=False, stop=True,
                        )
                    nc.vector.tensor_scalar(
                        out=a_sb[:, i, csl],
                        in0=v_ps[:],
                        scalar1=0.0,
                        scalar2=ctxv[:, i:i + 1],
                        op0=OP.max,
                        op1=OP.mult,
                    )

            # ---- P phase: output projection + bias + store ----
            # Output staged in half-rows [128, 2048] so each o-tile needs only
            # 2 dma_starts (issue time on Sync is the scarce resource).
            HSTG = N // 2
            for o in range(CT):
                for half in range(2):
                    o_sb = opool.tile([128, HSTG], FP, tag="osb", name="o_sb")
                    for hc in range(NCH // 2):
                        chk = half * (NCH // 2) + hc
                        p_ps = ps_mm.tile([128, FREE], FP, tag="mm512", name="p_ps")
                        csl = slice(chk * FREE, (chk + 1) * FREE)
                        for c2 in range(CT):
                            nc.tensor.matmul(
                                p_ps[:],
                                wp_sb[:, c2, o * 128:(o + 1) * 128],
                                a_sb[:, c2, csl],
                                start=(c2 == 0), stop=(c2 == CT - 1),
                            )
                        osl = slice(hc * FREE, (hc + 1) * FREE)
                        # Alternate evictions DVE/ScalarE to split the load.
                        if has_p_bias:
                            if chk % 2 == 0:
                                nc.vector.tensor_scalar_add(
                                    o_sb[:, osl], in0=p_ps[:],
                                    scalar1=bp_sb[:, o:o + 1],
                                )
                            else:
                                nc.scalar.add(
                                    o_sb[:, osl], p_ps[:], add=bp_sb[:, o:o + 1]
                                )
                        else:
                            if chk % 2 == 0:
                                nc.vector.tensor_copy(o_sb[:, osl], p_ps[:])
                            else:
                                nc.scalar.copy(o_sb[:, osl], p_ps[:])
                    ysl = y_d[b, o * 128:(o + 1) * 128,
                              half * HSTG:(half + 1) * HSTG]
                    if b == BPC - 1 and o == CT - 1:
                        # Final tile: store per chunk so the last DMA is
                        # small — shortens the kernel tail.
                        QS = FREE
                        for qs in range(HSTG // QS):
                            nc.sync.dma_start(
                                out=ysl[:, qs * QS:(qs + 1) * QS],
                                in_=o_sb[:, qs * QS:(qs + 1) * QS],
                            )
                    else:
                        nc.sync.dma_start(out=ysl, in_=o_sb[:])

    nc.compile()
    return nc


_NC_CACHE = {}


def kernel(x, Wqkv, bqkv, Wp, bp):
    global LAST_RESULTS
    x = np.ascontiguousarray(np.asarray(x, dtype=np.float32))
    Wqkv = np.asarray(Wqkv, dtype=np.float32)
    bqkv = np.asarray(bqkv, dtype=np.float32)
    Wp = np.asarray(Wp, dtype=np.float32)
    bp = np.asarray(bp, dtype=np.float32)

    # Host-side weight layout prep (tiny, one-time).
    bf16 = ml_dtypes.bfloat16
    r = Wqkv.reshape(H, 1 + 2 * HD, C)
    wqT = np.ascontiguousarray(r[:, 0, :].T).astype(bf16)             # [C, H]
    wkT = np.ascontiguousarray(r[:, 1:1 + HD, :].reshape(C, C).T).astype(bf16)
    wvT = np.ascontiguousarray(r[:, 1 + HD:, :].reshape(C, C).T).astype(bf16)
    wpT = np.ascontiguousarray(Wp.T).astype(bf16)                     # [C, o]
    rb = bqkv.reshape(H, 1 + 2 * HD)
    bq = np.ascontiguousarray(rb[:, 0].reshape(1, H)).astype(bf16)
    bk = np.ascontiguousarray(rb[:, 1:1 + HD].reshape(1, C)).astype(bf16)
    bv = np.ascontiguousarray(rb[:, 1 + HD:].reshape(1, C)).astype(bf16)
    maskhd = np.zeros((C, H), dtype=np.float32)
    for ch in range(C):
        maskhd[ch, ch // HD] = 1.0
    esel = np.zeros((128, H), dtype=np.float32)
    for p in range(128):
        if p % 32 < H:
            esel[p, p % 32] = 1.0

    has_qkv_bias = bool(np.any(bqkv != 0.0))
    has_p_bias = bool(np.any(bp != 0.0))

    key = (has_qkv_bias, has_p_bias)
    if key not in _NC_CACHE:
        _NC_CACHE[key] = _build(*key)
    nc = _NC_CACHE[key]

    shared = {
        "wkT": wkT, "wvT": wvT, "wpT": wpT, "wqT": wqT,
        "maskhd": maskhd, "esel": esel, "bq": bq, "bk": bk, "bv": bv, "bp": bp,
    }
    in_maps = [
        {"x": x[i * BPC:(i + 1) * BPC], **shared} for i in range(NCORES)
    ]
    LAST_RESULTS = run_bass_kernel_spmd(nc, in_maps, list(range(NCORES)))
    out = np.concatenate(
        [LAST_RESULTS.results[i]["y"] for i in range(NCORES)], axis=0
    )
    return out.astype(np.float32)


if __name__ == "__main__":
    rng = np.random.default_rng(0)
    x = rng.standard_normal((B, C, N), dtype=np.float32)
    Wqkv = (rng.standard_normal((H * (1 + 2 * HD), C), dtype=np.float32) * 0.02)
    bqkv = np.zeros((H * (1 + 2 * HD),), np.float32)
    Wp = rng.standard_normal((C, C), dtype=np.float32) * 0.02
    bp = np.zeros((C,), np.float32)
    y = kernel(x, Wqkv, bqkv, Wp, bp)
    print("out", y.shape, y.dtype)


# revision 53
# speedup vs baseline: 1.0077x; 1.0077x over previous
"""BroadcastAttention Trainium2 kernel (8 NeuronCores, data-parallel over batch).

Math per sample (C=512, N=4096, H=8 heads, HD=64):
    qkv = Wqkv @ x            # [H*(1+2HD), N]
    q[h,n], k[h,d,n], v[h,d,n] split per head
    s = softmax(q over n)     # [H, N]
    ctx[h,d] = sum_n k[h,d,n]*s[h,n]
    out = Wp @ (relu(v)*ctx) + bp

Device formulation (per core: 2 samples):
    - kT[n, kch] and expT[n, h] = exp(qT[n, h]) computed directly in
      [n-partition] orientation (softmax max-shift is unnecessary: |q|<~3,
      exp is safely in fp32 range; exp(q)/sum(exp q) == softmax exactly).
    - ctx2[h, kch] = sum_n expT[n,h] * kT[n,kch] via PE matmul accumulation;
      Z[h] = sum_n expT[n,h] via matmul with ones; ctx = ctx2 * (1/Z).
    - ctx transposed to [kch-partition] via PE transpose, head-selected with a
      0/1 mask, then fused into the PSUM->SBUF eviction of v as
      a = max(v,0)*ctx (DVE tensor_scalar).
    - proj matmul over a, bias-add on ScalarE during PSUM->SBUF eviction.

Matmul operands are bf16 (fp32 streams through the PE at half rate; measured
550ns vs ~220ns per 512-col matmul), accumulation in fp32 PSUM.
"""

import sys

for _p in ("/opt/trn_rl_repo",):
    if _p not in sys.path:
        sys.path.insert(0, _p)

from contextlib import ExitStack

import ml_dtypes
import numpy as np

import concourse.bass as bass
import concourse.mybir as mybir
import concourse.tile as tile
from concourse import bacc
from concourse.bass_utils import run_bass_kernel_spmd
from concourse.masks import make_identity

# Problem constants (hardcoded per contract; kernel.py must be self-contained).
B, C, N = 16, 512, 4096
H, HD = 8, 64
NCORES = 8
BPC = B // NCORES  # samples per core
CT = C // 128      # 4 contraction/partition tiles of 128
NT = N // 128      # 32 n-tiles (kT orientation)
FREE = 512         # matmul moving free-dim chunk
NCH = N // FREE    # 8 chunks
FP = mybir.dt.float32
BF = mybir.dt.bfloat16  # matmul operand dtype (fp32 PSUM accumulation)

# Results of the last run (for test harness introspection).
LAST_RESULTS = None


def _build(has_qkv_bias: bool, has_p_bias: bool) -> bass.Bass:
    nc = bacc.Bacc("TRN2", target_bir_lowering=False, debug=False)

    x_d = nc.declare_dram_parameter("x", [BPC, C, N], FP, isOutput=False)
    wk_d = nc.declare_dram_parameter("wkT", [C, C], BF, isOutput=False)
    wv_d = nc.declare_dram_parameter("wvT", [C, C], BF, isOutput=False)
    wp_d = nc.declare_dram_parameter("wpT", [C, C], BF, isOutput=False)
    wq_d = nc.declare_dram_parameter("wqT", [C, H], BF, isOutput=False)
    mask_d = nc.declare_dram_parameter("maskhd", [C, H], FP, isOutput=False)
    esel_d = nc.declare_dram_parameter("esel", [128, H], FP, isOutput=False)
    bq_d = nc.declare_dram_parameter("bq", [1, H], BF, isOutput=False)
    bk_d = nc.declare_dram_parameter("bk", [1, C], BF, isOutput=False)
    bv_d = nc.declare_dram_parameter("bv", [1, C], BF, isOutput=False)
    bp_d = nc.declare_dram_parameter("bp", [C], FP, isOutput=False)
    y_d = nc.declare_dram_parameter("y", [BPC, C, N], FP, isOutput=True)

    AF = mybir.ActivationFunctionType
    OP = mybir.AluOpType

    with tile.TileContext(nc) as tc, ExitStack() as ctx:
        consts = ctx.enter_context(tc.tile_pool(name="consts", bufs=1))
        xstage = ctx.enter_context(tc.tile_pool(name="xstage", bufs=4))
        xpool = ctx.enter_context(tc.tile_pool(name="xpool", bufs=2))
        apool = ctx.enter_context(tc.tile_pool(name="apool", bufs=1))
        spool = ctx.enter_context(tc.tile_pool(name="spool", bufs=2))
        ktpool = ctx.enter_context(tc.tile_pool(name="ktpool", bufs=10))
        opool = ctx.enter_context(tc.tile_pool(name="opool", bufs=3))
        small = ctx.enter_context(tc.tile_pool(name="small", bufs=2))
        ps_mm = ctx.enter_context(tc.tile_pool(name="ps_mm", bufs=4, space="PSUM"))
        ps_q = ctx.enter_context(tc.tile_pool(name="ps_q", bufs=2, space="PSUM"))
        ps_ctx = ctx.enter_context(tc.tile_pool(name="ps_ctx", bufs=1, space="PSUM"))
        ps_z = ctx.enter_context(tc.tile_pool(name="ps_z", bufs=1, space="PSUM"))

        # ---- constants / weights into SBUF ----
        wk_sb = consts.tile([128, CT, C], BF)
        wv_sb = consts.tile([128, CT, C], BF)
        wp_sb = consts.tile([128, CT, C], BF)
        wq_sb = consts.tile([128, CT, H], BF)
        mask_sb = consts.tile([128, CT, H], FP)
        bp_sb = consts.tile([128, CT], FP)
        # Only the QK-phase weights load up front (GpSimd queue, away from x
        # on Sync). wv/wp/mask/bp are not needed until the V/P phases —
        # issuing them on Sync *after* batch 0's x chunks keeps the first
        # ~25us of HBM bandwidth for x (the startup critical path).
        for ct in range(CT):
            sl = slice(ct * 128, (ct + 1) * 128)
            nc.gpsimd.dma_start(out=wk_sb[:, ct, :], in_=wk_d[sl, :])
            nc.gpsimd.dma_start(out=wq_sb[:, ct, :], in_=wq_d[sl, :])

        identity = consts.tile([128, 128], FP)
        make_identity(nc, identity[:])
        ones_col = consts.tile([128, 1], FP)
        nc.vector.memset(ones_col[:], 1.0)
        esel_sb = consts.tile([128, H], FP)
        nc.gpsimd.dma_start(out=esel_sb[:], in_=esel_d[:, :])

        if has_qkv_bias:
            bq_sb = consts.tile([1, H], BF)
            bk_sb = consts.tile([1, C], BF)
            bv_sb = consts.tile([1, C], BF)
            ones_row = consts.tile([1, FREE], BF)
            nc.sync.dma_start(out=bq_sb[:], in_=bq_d[:, :])
            nc.sync.dma_start(out=bk_sb[:], in_=bk_d[:, :])
            nc.sync.dma_start(out=bv_sb[:], in_=bv_d[:, :])
            nc.vector.memset(ones_row[:], 1.0)

        for b in range(BPC):
            # ---- load x[b] (fp32) and convert to bf16 ----
            # Load x in n-quarters (quarter-major) so the QK loop can start
            # after ~1/4 of the 8.4MB load instead of all of it; conversions
            # alternate ScalarE/VectorE so neither FIFO stalls the QK path.
            # All conversions live on ScalarE: DVE is the busier engine
            # (PSUM evictions), and ScalarE has slack.
            x_sb = xpool.tile([128, CT, N], BF, tag="x_sb", name="x_sb")
            NQ = 8 if b == 0 else 4  # fine chunks only where start latency matters
            QN = N // NQ
            for qd in range(NQ):
                nsl = slice(qd * QN, (qd + 1) * QN)
                for ct in range(CT):
                    xst = xstage.tile([128, QN], FP, tag="xst", name="xst")
                    nc.sync.dma_start(
                        out=xst[:], in_=x_d[b, ct * 128:(ct + 1) * 128, nsl]
                    )
                    if (qd * CT + ct) % 2 == 0:
                        nc.scalar.copy(x_sb[:, ct, nsl], xst[:])
                    else:
                        nc.vector.tensor_copy(x_sb[:, ct, nsl], xst[:])
            if b == 0:
                for ct in range(CT):
                    sl = slice(ct * 128, (ct + 1) * 128)
                    nc.sync.dma_start(out=wv_sb[:, ct, :], in_=wv_d[sl, :])
                    nc.sync.dma_start(out=wp_sb[:, ct, :], in_=wp_d[sl, :])
                    nc.sync.dma_start(out=mask_sb[:, ct, :], in_=mask_d[sl, :])
                nc.sync.dma_start(
                    out=bp_sb[:], in_=bp_d.rearrange("(o p) -> p o", p=128)
                )

            # ---- QK phase: kT / expT per n-tile, ctx & Z accumulation ----
            # ctx matmuls have M=8 (15/16 of the PE array idle), so they are
            # issued back-to-back in groups of 4 at distinct 32-column groups
            # (tile_position) to run concurrently; ctx_big[32j+h] holds the
            # partial sum for sub-stream j, combined afterwards with one
            # matmul against the constant selection matrix esel.
            scoresT = spool.tile([128, NT, H], BF, tag="scoresT", name="scoresT")
            ctx_big = ps_ctx.tile([128, C], FP, tag="ctx", name="ctx_big")
            # Zero the whole bank: the M=8 col-tiled matmuls only write rows
            # 32j..32j+7, but the esel combine matmul reads all 128 rows.
            nc.vector.memset(ctx_big[:], 0.0)
            def emit_ctx_group(ntg, kts):
                for j in range(4):
                    nt = ntg * 4 + j
                    nc.tensor.matmul(
                        ctx_big[32 * j:32 * j + H, :],
                        scoresT[:, nt, :], kts[j][:],
                        start=False, stop=(ntg == NT // 4 - 1),
                        skip_group_check=True,
                        tile_position=(0, 32 * j),
                    )

            # ctx matmul groups are issued one group late (software pipeline)
            # so their kt/scoresT inputs are already evicted from PSUM by the
            # time PE reaches them — no PE stall on the DVE copies.
            prev = None
            for ntg in range(NT // 4):
                kts = []
                for j in range(4):
                    nt = ntg * 4 + j
                    k_ps = ps_mm.tile([128, C], FP, tag="mm512", name="k_ps")
                    q_ps = ps_q.tile([128, H], FP, tag="q8", name="q_ps")
                    nsl = slice(nt * 128, (nt + 1) * 128)
                    for ct in range(CT):
                        xsl = x_sb[:, ct, nsl]
                        last = (ct == CT - 1) and not has_qkv_bias
                        nc.tensor.matmul(
                            k_ps[:], xsl, wk_sb[:, ct, :],
                            start=(ct == 0), stop=last,
                        )
                        nc.tensor.matmul(
                            q_ps[:], xsl, wq_sb[:, ct, :],
                            start=(ct == 0), stop=last,
                        )
                    if has_qkv_bias:
                        onesl = ones_row[:, 0:128]
                        nc.tensor.matmul(k_ps[:], onesl, bk_sb[:, :], start=False, stop=True)
                        nc.tensor.matmul(q_ps[:], onesl, bq_sb[:, :], start=False, stop=True)
                    st = scoresT[:, nt, :]
                    nc.scalar.activation(out=st, in_=q_ps[:], func=AF.Exp)
                    kt_sb = ktpool.tile([128, C], BF, tag="kt", name="kt_sb")
                    nc.vector.tensor_copy(out=kt_sb[:], in_=k_ps[:])
                    kts.append(kt_sb)
                if prev is not None:
                    emit_ctx_group(ntg - 1, prev)
                prev = kts
            emit_ctx_group(NT // 4 - 1, prev)

            # ---- ctx finalize: combine sub-streams, normalize, transpose,
            # head-select ----
            ctxcopy = small.tile([128, C], FP, tag="ctxcopy", name="ctxcopy")
            nc.vector.tensor_copy(out=ctxcopy[:], in_=ctx_big[:])
            ctx2_ps = ps_q.tile([H, C], FP, tag="q8", name="ctx2_ps")
            nc.tensor.matmul(ctx2_ps[:], esel_sb[:], ctxcopy[:], start=True, stop=True)
            # Z[h] = sum_n exp(q): per-partition partial on DVE, then one tiny
            # cross-partition matmul (cheaper than 64 PE matmuls).
            zpart = small.tile([128, H], FP, tag="zpart", name="zpart")
            nc.vector.reduce_sum(
                out=zpart[:],
                in_=scoresT[:].rearrange("p nt h -> p h nt"),
                axis=mybir.AxisListType.X,
            )
            z_ps = ps_z.tile([H, 1], FP, tag="z", name="z_ps")
            nc.tensor.matmul(z_ps[:], zpart[:], ones_col[:], start=True, stop=True)
            invz = small.tile([H, 1], FP, tag="invz", name="invz")
            nc.vector.reciprocal(out=invz[:], in_=z_ps[:])
            ctxn = small.tile([H, C], FP, tag="ctxn", name="ctxn")
            nc.vector.tensor_scalar_mul(out=ctxn[:], in0=ctx2_ps[:], scalar1=invz[:])
            ctxv = small.tile([128, CT], FP, tag="ctxv", name="ctxv")
            for i in range(CT):
                ctxT_ps = ps_q.tile([128, H], FP, tag="q8", name="ctxT_ps")
                nc.tensor.transpose(
                    ctxT_ps[:], ctxn[:, i * 128:(i + 1) * 128], identity[:H, :H]
                )
                junk = small.tile([128, H], FP, tag="junk", name="junk")
                nc.vector.tensor_tensor(
                    out=junk[:], in0=ctxT_ps[:], in1=mask_sb[:, i, :], op=OP.mult
                )
                nc.vector.reduce_sum(
                    out=ctxv[:, i:i + 1], in_=junk[:], axis=mybir.AxisListType.X
                )

            # ---- V phase: v matmul + fused relu*ctx eviction ----
            a_sb = apool.tile([128, CT, N], BF, tag="a_sb", name="a_sb")
            for i in range(CT):
                for chk in range(NCH):
                    v_ps = ps_mm.tile([128, FREE], FP, tag="mm512", name="v_ps")
                    csl = slice(chk * FREE, (chk + 1) * FREE)
                    for ct in range(CT):
                        last = (ct == CT - 1) and not has_qkv_bias
                        nc.tensor.matmul(
                            v_ps[:],
                            wv_sb[:, ct, i * 128:(i + 1) * 128],
                            x_sb[:, ct, csl],
                            start=(ct == 0), stop=last,
                        )
                    if has_qkv_bias:
                        nc.tensor.matmul(
                            v_ps[:], bv_sb[:, i * 128:(i + 1) * 128], ones_row[:],
                            start=False, stop=True,
                        )
                    nc.vector.tensor_scalar(
                        out=a_sb[:, i, csl],
                        in0=v_ps[:],
                        scalar1=0.0,
                        scalar2=ctxv[:, i:i + 1],
                        op0=OP.max,
                        op1=OP.mult,
                    )

            # ---- P phase: output projection + bias + store ----
            # Output staged in half-rows [128, 2048] so each o-tile needs only
            # 2 dma_starts (issue time on Sync is the scarce resource).
            HSTG = N // 2
            for o in range(CT):
                for half in range(2):
                    o_sb = opool.tile([128, HSTG], FP, tag="osb", name="o_sb")
                    for hc in range(NCH // 2):
                        chk = half * (NCH // 2) + hc
                        p_ps = ps_mm.tile([128, FREE], FP, tag="mm512", name="p_ps")
                        csl = slice(chk * FREE, (chk + 1) * FREE)
                        for c2 in range(CT):
                            nc.tensor.matmul(
                                p_ps[:],
                                wp_sb[:, c2, o * 128:(o + 1) * 128],
                                a_sb[:, c2, csl],
                                start=(c2 == 0), stop=(c2 == CT - 1),
                            )
                        osl = slice(hc * FREE, (hc + 1) * FREE)
                        # Alternate evictions DVE/ScalarE to split the load.
                        if has_p_bias:
                            if chk % 2 == 0:
                                nc.vector.tensor_scalar_add(
                                    o_sb[:, osl], in0=p_ps[:],
                                    scalar1=bp_sb[:, o:o + 1],
                                )
                            else:
                                nc.scalar.add(
                                    o_sb[:, osl], p_ps[:], add=bp_sb[:, o:o + 1]
                                )
                        else:
                            if chk % 2 == 0:
                                nc.vector.tensor_copy(o_sb[:, osl], p_ps[:])
                            else:
                                nc.scalar.copy(o_sb[:, osl], p_ps[:])
                    ysl = y_d[b, o * 128:(o + 1) * 128,
                              half * HSTG:(half + 1) * HSTG]
                    if b == BPC - 1 and o == CT - 1:
                        # Final tile: store per chunk so the last DMA is
                        # small — shortens the kernel tail.
                        QS = FREE
                        for qs in range(HSTG // QS):
                            nc.sync.dma_start(
                                out=ysl[:, qs * QS:(qs + 1) * QS],
                                in_=o_sb[:, qs * QS:(qs + 1) * QS],
                            )
                    else:
                        nc.sync.dma_start(out=ysl, in_=o_sb[:])

    nc.compile()
    return nc


_NC_CACHE = {}


def kernel(x, Wqkv, bqkv, Wp, bp):
    global LAST_RESULTS
    x = np.ascontiguousarray(np.asarray(x, dtype=np.float32))
    Wqkv = np.asarray(Wqkv, dtype=np.float32)
    bqkv = np.asarray(bqkv, dtype=np.float32)
    Wp = np.asarray(Wp, dtype=np.float32)
    bp = np.asarray(bp, dtype=np.float32)

    # Host-side weight layout prep (tiny, one-time).
    bf16 = ml_dtypes.bfloat16
    r = Wqkv.reshape(H, 1 + 2 * HD, C)
    wqT = np.ascontiguousarray(r[:, 0, :].T).astype(bf16)             # [C, H]
    wkT = np.ascontiguousarray(r[:, 1:1 + HD, :].reshape(C, C).T).astype(bf16)
    wvT = np.ascontiguousarray(r[:, 1 + HD:, :].reshape(C, C).T).astype(bf16)
    wpT = np.ascontiguousarray(Wp.T).astype(bf16)                     # [C, o]
    rb = bqkv.reshape(H, 1 + 2 * HD)
    bq = np.ascontiguousarray(rb[:, 0].reshape(1, H)).astype(bf16)
    bk = np.ascontiguousarray(rb[:, 1:1 + HD].reshape(1, C)).astype(bf16)
    bv = np.ascontiguousarray(rb[:, 1 + HD:].reshape(1, C)).astype(bf16)
    maskhd = np.zeros((C, H), dtype=np.float32)
    for ch in range(C):
        maskhd[ch, ch // HD] = 1.0
    esel = np.zeros((128, H), dtype=np.float32)
    for p in range(128):
        if p % 32 < H:
            esel[p, p % 32] = 1.0

    has_qkv_bias = bool(np.any(bqkv != 0.0))
    has_p_bias = bool(np.any(bp != 0.0))

    key = (has_qkv_bias, has_p_bias)
    if key not in _NC_CACHE:
        _NC_CACHE[key] = _build(*key)
    nc = _NC_CACHE[key]

    shared = {
        "wkT": wkT, "wvT": wvT, "wpT": wpT, "wqT": wqT,
        "maskhd": maskhd, "esel": esel, "bq": bq, "bk": bk, "bv": bv, "bp": bp,
    }
    in_maps = [
        {"x": x[i * BPC:(i + 1) * BPC], **shared} for i in range(NCORES)
    ]
    LAST_RESULTS = run_bass_kernel_spmd(nc, in_maps, list(range(NCORES)))
    out = np.concatenate(
        [LAST_RESULTS.results[i]["y"] for i in range(NCORES)], axis=0
    )
    return out.astype(np.float32)


if __name__ == "__main__":
    rng = np.random.default_rng(0)
    x = rng.standard_normal((B, C, N), dtype=np.float32)
    Wqkv = (rng.standard_normal((H * (1 + 2 * HD), C), dtype=np.float32) * 0.02)
    bqkv = np.zeros((H * (1 + 2 * HD),), np.float32)
    Wp = rng.standard_normal((C, C), dtype=np.float32) * 0.02
    bp = np.zeros((C,), np.float32)
    y = kernel(x, Wqkv, bqkv, Wp, bp)
    print("out", y.shape, y.dtype)
